# revision 27
# baseline (speedup 1.0000x reference)
"""Trainium2 Bass kernel for MeshGenLoss (Chamfer + KL + density-uniformity).

Math: d[i,j] = |a_i|^2 + |b_j|^2 - 2 a_i.b_j as ONE K=33 bf16 matmul per
[128,512] tile (3 exact bf16 limbs per fp32 scalar -> fp32-exact distances
in PSUM at bf16 matmul speed).

v4 structure ("negated world" + banded pp):
 - Everything evacuated from PSUM is NEGATED (ACT copy scale=-1, bf16), so
   all min-reductions become MAX ops; host negates at the end.
 - pt (pred x target): full [512 x 4096] rows per core. Row-maxes (pred-side
   Chamfer) via DVE bf16 TT-fold chains; column-maxes (target-side) via
   slab max-merge -> DMA transpose -> 3D fold -> reduce.
 - pp (pred self-distance) uses SYMMETRY HARDER than v3: each 128-row block
   g computes only col-blocks (g+k) mod 32 for k in 0..16 (17 blocks = 2176
   cols instead of 4096). Every unordered pair lands in the band of one of
   its two blocks, so nn_pp = min(band row-mins, band col-mins). This cuts
   pp matmul + evacuation + merge volume by ~47% at the cost of adding pp
   row-min folds (cheap, bf16 2x).
 - pp slab merges run on GpSimd (idle engine) to unload DVE.
 - Diagonal mask: (1000*I)^T@(1000*I) accumulated into the first 128 cols
   of each pp row-block's chunk0 (band starts at the diagonal).

Sharding: core c owns pred/target rows [512c, 512c+512). rhs_p columns are
host-rotated by -512c and sliced to [0,2560) so each row-block r's band is
the contiguous rotated cols [128r, 128r+2176) (identical SPMD program).
"""

import sys

import ml_dtypes
import numpy as np

sys.path.insert(0, "/opt/trn_rl_repo")

B = 2
N = 4096
L = 512
CORES = 8
ROWS = N // CORES  # 512 rows per core
RB = ROWS // 128  # 4 row blocks per core
K = 33
BF16 = ml_dtypes.bfloat16
NBLK = N // 128  # 32 global 128-blocks
BANDB = 17  # col blocks per pp row-block band
BANDW = BANDB * 128  # 2176
PPW = (RB - 1) * 128 + BANDW  # 2560 rotated pp cols touched per core
PPB = PPW // 128  # 20

NEG_BIG = -3.0e38

# per-batch job order: seeds first, pt slab completes early (transpose
# overlaps pp2/pp3), pp slab last (shorter finish chain).
JOB_ORDER = [("pt", 0), ("pp", 0), ("pt", 1), ("pp", 1),
             ("pt", 2), ("pt", 3), ("pp", 2), ("pp", 3)]


def _limbs3(x):
    """Split float64 array into 3 bf16 limbs capturing ~24 significand bits."""
    h = x.astype(BF16)
    r = x - h.astype(np.float64)
    m = r.astype(BF16)
    r2 = r - m.astype(np.float64)
    lo = r2.astype(BF16)
    return h, m, lo


def _build_lhsT(a):
    """a: [n, 3] float64 row points -> lhsT [33, n] bf16."""
    n = a.shape[0]
    asq = (a * a).sum(-1)
    al = _limbs3(a)
    sl = _limbs3(asq)
    out = np.zeros((K, n), dtype=BF16)
    k = 0
    for t in range(3):
        for p in range(3):
            row = (-2.0 * al[p][:, t].astype(np.float64)).astype(BF16)
            for _q in range(3):
                out[k] = row
                k += 1
    for p in range(3):
        out[k] = sl[p]
        k += 1
    for _q in range(3):
        out[k] = np.ones(n, dtype=BF16)
        k += 1
    return out


def _build_rhs(b):
    """b: [m, 3] float64 column points -> rhs [33, m] bf16."""
    m = b.shape[0]
    bsq = (b * b).sum(-1)
    bl = _limbs3(b)
    sl = _limbs3(bsq)
    out = np.zeros((K, m), dtype=BF16)
    k = 0
    for t in range(3):
        for _p in range(3):
            for q in range(3):
                out[k] = bl[q][:, t]
                k += 1
    for _p in range(3):
        out[k] = np.ones(m, dtype=BF16)
        k += 1
    for q in range(3):
        out[k] = sl[q]
        k += 1
    return out


def _build_program():
    import concourse.bacc as bacc
    import concourse.mybir as mybir
    import concourse.tile as tile
    from contextlib import ExitStack

    dt = mybir.dt
    Alu = mybir.AluOpType
    Act = mybir.ActivationFunctionType

    nc = bacc.Bacc("TRN2", target_bir_lowering=False, debug=False)

    d_lhsT = nc.declare_dram_parameter("lhsT", [B, K, ROWS], dt.bfloat16, isOutput=False)
    d_rhs_t = nc.declare_dram_parameter("rhs_t", [B, K, N], dt.bfloat16, isOutput=False)
    d_rhs_p = nc.declare_dram_parameter("rhs_p", [B, K, PPW], dt.bfloat16, isOutput=False)
    d_dql = nc.declare_dram_parameter("dql", [128, 128], dt.bfloat16, isOutput=False)
    d_mu = nc.declare_dram_parameter("mu_sl", [1, 128], dt.float32, isOutput=False)
    d_lv = nc.declare_dram_parameter("lv_sl", [1, 128], dt.float32, isOutput=False)

    # o_min: NEGATED row maxes; col 4b+r = pt, col 8+4b+r = pp band
    o_min = nc.declare_dram_parameter("o_min", [128, 16], dt.float32, isOutput=True)
    # o_ct: NEGATED col maxes: pt b: cols 32b..32b+32 (block t);
    #       pp b: cols 64+20b..64+20b+20 (rotated block p)
    o_ct = nc.declare_dram_parameter("o_ct", [128, 104], dt.float32, isOutput=True)
    o_kl = nc.declare_dram_parameter("o_kl", [1, 3], dt.float32, isOutput=True)

    with tile.TileContext(nc) as tc, ExitStack() as ctx:
        consts = ctx.enter_context(tc.tile_pool(name="consts", bufs=1))
        psum = ctx.enter_context(tc.tile_pool(name="psum", bufs=2, space="PSUM"))
        stpool = ctx.enter_context(tc.tile_pool(name="st", bufs=3))
        utpool = ctx.enter_context(tc.tile_pool(name="ut", bufs=2))
        fpool = ctx.enter_context(tc.tile_pool(name="f", bufs=2))

        # ---- resident inputs --------------------------------------------
        lhsT_sb = {}
        rhs_sb = {}
        for b in range(B):
            t1 = consts.tile([K, ROWS], dt.bfloat16, tag=f"l{b}")
            rt = consts.tile([K, N], dt.bfloat16, tag=f"rt{b}")
            rp = consts.tile([K, PPW], dt.bfloat16, tag=f"rp{b}")
            lhsT_sb[b] = t1
            rhs_sb["pt", b] = rt
            rhs_sb["pp", b] = rp
        dql_sb = consts.tile([128, 128], dt.bfloat16, tag="dql")
        mu_sb = consts.tile([1, 128], dt.float32, tag="mu")
        lv_sb = consts.tile([1, 128], dt.float32, tag="lv")

        # input DMAs: the first matmul needs lhsT[0] + rhs_t[0][:, :512] —
        # issue those in PARALLEL on different queues; bulk follows.
        nc.sync.dma_start(out=lhsT_sb[0][:], in_=d_lhsT[0])
        nc.gpsimd.dma_start(out=rhs_sb["pt", 0][:, :512], in_=d_rhs_t[0, :, :512])
        nc.gpsimd.dma_start(out=dql_sb[:], in_=d_dql[:])
        nc.gpsimd.dma_start(out=rhs_sb["pt", 0][:, 512:2048], in_=d_rhs_t[0, :, 512:2048])
        nc.sync.dma_start(out=rhs_sb["pt", 0][:, 2048:], in_=d_rhs_t[0, :, 2048:])
        nc.gpsimd.dma_start(out=rhs_sb["pp", 0][:, :1280], in_=d_rhs_p[0, :, :1280])
        nc.sync.dma_start(out=rhs_sb["pp", 0][:, 1280:], in_=d_rhs_p[0, :, 1280:])
        nc.sync.dma_start(out=lhsT_sb[1][:], in_=d_lhsT[1])
        nc.gpsimd.dma_start(out=rhs_sb["pt", 1][:, :2048], in_=d_rhs_t[1, :, :2048])
        nc.sync.dma_start(out=rhs_sb["pt", 1][:, 2048:], in_=d_rhs_t[1, :, 2048:])
        nc.gpsimd.dma_start(out=rhs_sb["pp", 1][:], in_=d_rhs_p[1])
        nc.gpsimd.dma_start(out=mu_sb[:], in_=d_mu[:])
        nc.gpsimd.dma_start(out=lv_sb[:], in_=d_lv[:])

        omin_sb = consts.tile([128, 16], dt.float32, tag="omin")
        ctall = consts.tile([128, 104], dt.float32, tag="ctall")

        # persistent slabs (negated, max-merged)
        slab = {}
        for b in range(B):
            spt = consts.tile([128, N], dt.bfloat16, tag=f"spt{b}")
            spp = consts.tile([128, PPW], dt.bfloat16, tag=f"spp{b}")
            slab["pt", b] = spt
            slab["pp", b] = spp

        def finish_slab(kind, b):
            """slab -> DMA transpose -> 3D TT-fold -> reduce into ctall."""
            if kind == "pt":
                nb, off = NBLK, 32 * b
            else:
                nb, off = PPB, 64 + PPB * b
            ut = utpool.tile([128, nb, 128], dt.bfloat16, tag=f"ut{kind}")
            nc.sync.dma_start_transpose(ut[:], slab[kind, b][:])
            g1 = fpool.tile([128, nb, 64], dt.bfloat16, tag=f"g1{kind}")
            nc.vector.tensor_tensor(g1[:], ut[:, :, 0:64], ut[:, :, 64:128], Alu.max)
            g2 = fpool.tile([128, nb, 32], dt.bfloat16, tag=f"g2{kind}")
            nc.vector.tensor_tensor(g2[:], g1[:, :, 0:32], g1[:, :, 32:64], Alu.max)
            g3 = fpool.tile([128, nb, 16], dt.bfloat16, tag=f"g3{kind}")
            nc.vector.tensor_tensor(g3[:], g2[:, :, 0:16], g2[:, :, 16:32], Alu.max)
            nc.vector.tensor_reduce(
                ctall[:, off:off + nb], g3[:], axis=mybir.AxisListType.X, op=Alu.max)
            nc.sync.dma_start(out=o_ct[:, off:off + nb], in_=ctall[:, off:off + nb])

        for b in range(B):
            for kind, r in JOB_ORDER:
                lhsT = lhsT_sb[b][:, 128 * r:128 * (r + 1)]
                rhs = rhs_sb[kind, b]
                if kind == "pt":
                    base, w1 = 0, 2048  # second chunk width
                else:
                    base, w1 = 128 * r, 128
                if r == 0:
                    st = slab[kind, b]
                else:
                    st = stpool.tile([128, N if kind == "pt" else BANDW],
                                     dt.bfloat16, tag=f"st{kind}")
                # ---- chunk 0: [128, 2048] = 4 matmuls (+ diag for pp) ----
                ch0 = psum.tile([128, 2048], dt.float32, tag="ps")
                if kind == "pp":
                    # diag block [0:128] is its own accumulation group
                    nc.tensor.matmul(
                        ch0[:, 0:128], lhsT, rhs[:, base:base + 128],
                        start=True, stop=False)
                    nc.tensor.matmul(
                        ch0[:, 0:128], dql_sb[:], dql_sb[:],
                        start=False, stop=True)
                    nc.tensor.matmul(
                        ch0[:, 128:512], lhsT, rhs[:, base + 128:base + 512],
                        start=True, stop=True)
                    t0 = 1
                else:
                    t0 = 0
                for t in range(t0, 4):
                    c0 = base + 512 * t
                    nc.tensor.matmul(
                        ch0[:, 512 * t:512 * (t + 1)], lhsT, rhs[:, c0:c0 + 512],
                        start=True, stop=True)
                nc.scalar.activation(st[:, 0:2048], ch0[:], Act.Copy, scale=-1.0)
                # ---- chunk 1: pt [128,2048], pp [128,128] ----------------
                ch1 = psum.tile([128, 2048], dt.float32, tag="ps")
                nmm = w1 // 512 if w1 >= 512 else 1
                for t in range(nmm):
                    c0 = base + 2048 + 512 * t
                    cw = min(512, w1 - 512 * t)
                    nc.tensor.matmul(
                        ch1[:, 512 * t:512 * t + cw], lhsT, rhs[:, c0:c0 + cw],
                        start=True, stop=True)
                nc.scalar.activation(
                    st[:, 2048:2048 + w1], ch1[:, 0:w1], Act.Copy, scale=-1.0)
                # ---- row-max fold chain (TT bf16 2x) + final reduce ------
                col = 4 * b + r if kind == "pt" else 8 + 4 * b + r
                if kind == "pt":
                    f1 = fpool.tile([128, 2048], dt.bfloat16, tag="f1")
                    nc.vector.tensor_tensor(
                        f1[:], st[:, 0:2048], st[:, 2048:4096], Alu.max)
                    f2 = fpool.tile([128, 1024], dt.bfloat16, tag="f2")
                    nc.vector.tensor_tensor(
                        f2[:], f1[:, 0:1024], f1[:, 1024:2048], Alu.max)
                    f3 = fpool.tile([128, 512], dt.bfloat16, tag="f3")
                    nc.vector.tensor_tensor(
                        f3[:], f2[:, 0:512], f2[:, 512:1024], Alu.max)
                    nc.vector.tensor_reduce(
                        omin_sb[:, col:col + 1], f3[:],
                        axis=mybir.AxisListType.X, op=Alu.max)
                else:
                    q1 = fpool.tile([128, 1088], dt.bfloat16, tag="q1")
                    nc.vector.tensor_tensor(
                        q1[:], st[:, 0:1088], st[:, 1088:2176], Alu.max)
                    q2 = fpool.tile([128, 544], dt.bfloat16, tag="q2")
                    nc.vector.tensor_tensor(
                        q2[:], q1[:, 0:544], q1[:, 544:1088], Alu.max)
                    nc.vector.tensor_reduce(
                        omin_sb[:, col:col + 1], q2[:],
                        axis=mybir.AxisListType.X, op=Alu.max)
                # ---- slab merge ------------------------------------------
                if kind == "pp" and r == 0:
                    # init the slab tail the r=0 band doesn't reach
                    nc.vector.memset(slab["pp", b][:, BANDW:PPW], NEG_BIG)
                if r > 0:
                    if kind == "pt":
                        nc.vector.tensor_tensor(
                            slab["pt", b][:], slab["pt", b][:], st[:], Alu.max)
                    else:
                        win = slab["pp", b][:, base:base + BANDW]
                        nc.vector.tensor_tensor(win, win, st[:], Alu.max)
                if (kind, r) in (("pt", 3), ("pp", 3)):
                    finish_slab(kind, b)
                if b == B - 1 and r == 3:
                    half = slice(0, 8) if kind == "pt" else slice(8, 16)
                    nc.sync.dma_start(out=o_min[:, half], in_=omin_sb[:, half])

        # ---- KL partials (at the end: avoids ACT table switches mid-
        # stream; Exp/Square live in a different table set than Copy) ----
        klt = consts.tile([1, 3], dt.float32, tag="klt")
        cp_t = consts.tile([1, 128], dt.float32, tag="klcp")
        nc.scalar.activation(cp_t[:], lv_sb[:], Act.Copy, accum_out=klt[:, 0:1])
        e_t = consts.tile([1, 128], dt.float32, tag="klexp")
        nc.scalar.activation(e_t[:], lv_sb[:], Act.Exp, accum_out=klt[:, 2:3])
        sq_t = consts.tile([1, 128], dt.float32, tag="klsq")
        nc.scalar.activation(sq_t[:], mu_sb[:], Act.Square, accum_out=klt[:, 1:2])
        nc.gpsimd.dma_start(out=o_kl[:], in_=klt[:])

    nc.compile()
    return nc


def _make_in_maps(pred, target, mu, logvar):
    pred = np.asarray(pred, dtype=np.float32)
    target = np.asarray(target, dtype=np.float32)
    mu = np.asarray(mu, dtype=np.float32)
    logvar = np.asarray(logvar, dtype=np.float32)

    pred64 = pred.astype(np.float64)
    target64 = target.astype(np.float64)

    rhs_t = np.stack([_build_rhs(target64[b]) for b in range(B)])  # [B,K,N]
    rhs_p_full = np.stack([_build_rhs(pred64[b]) for b in range(B)])
    dql = (np.eye(128) * 1000.0).astype(BF16)
    mu_flat = mu.reshape(-1)
    lv_flat = logvar.reshape(-1)

    in_maps = []
    for c in range(CORES):
        rows = slice(ROWS * c, ROWS * (c + 1))
        lhsT = np.stack([_build_lhsT(pred64[b, rows]) for b in range(B)])
        rot = np.roll(rhs_p_full, -ROWS * c, axis=2)[:, :, :PPW]
        in_maps.append({
            "lhsT": lhsT,
            "rhs_t": rhs_t,
            "rhs_p": np.ascontiguousarray(rot),
            "dql": dql,
            "mu_sl": mu_flat[128 * c:128 * (c + 1)].reshape(1, 128),
            "lv_sl": lv_flat[128 * c:128 * (c + 1)].reshape(1, 128),
        })
    return in_maps


def _host_post(results):
    # row mins (negated maxes): pt cols 0..7, pp cols 8..15
    nn_pt = np.empty((B, N), dtype=np.float64)
    pp_row = np.empty((B, N), dtype=np.float64)
    for c in range(CORES):
        om = -results[c]["o_min"].astype(np.float64)  # [128, 16]
        for b in range(B):
            for r in range(RB):
                rows = slice(ROWS * c + 128 * r, ROWS * c + 128 * r + 128)
                nn_pt[b, rows] = om[:, 4 * b + r]
                pp_row[b, rows] = om[:, 8 + 4 * b + r]

    # col mins: o_ct [128, 104] negated
    cts = -np.stack([r["o_ct"] for r in results]).astype(np.float64)
    nn_tp = np.full((B, N), np.inf)
    pp_col = np.full((B, N), np.inf)
    for c in range(CORES):
        for b in range(B):
            pt_v = cts[c][:, 32 * b:32 * b + 32]  # [jrel, t]
            nn_tp[b] = np.minimum(nn_tp[b], pt_v.T.reshape(N))
            pp_v = cts[c][:, 64 + PPB * b:64 + PPB * b + PPB]  # [jrel, p]
            for p in range(PPB):
                gblk = (4 * c + p) % NBLK
                cols = slice(128 * gblk, 128 * gblk + 128)
                pp_col[b, cols] = np.minimum(pp_col[b, cols], pp_v[:, p])
    nn_pp = np.minimum(pp_row, pp_col)

    kl_parts = np.stack([r["o_kl"].reshape(3) for r in results])

    cd = (nn_pt.mean(axis=1) + nn_tp.mean(axis=1)).mean()

    s1 = kl_parts[:, 0].astype(np.float64).sum()
    s2 = kl_parts[:, 1].astype(np.float64).sum()
    s3 = kl_parts[:, 2].astype(np.float64).sum()
    n_kl = B * L
    kl = -0.5 * (n_kl + s1 - s2 - s3) / n_kl

    density = np.std(nn_pp, axis=1, ddof=1).mean()

    total = cd + 0.001 * kl + 0.1 * density

    return (
        np.float32(total),
        np.float32(cd),
        np.float32(kl),
        np.float32(density),
    )


def kernel(pred, target, mu, logvar):
    from concourse.bass_utils import run_bass_kernel_spmd

    in_maps = _make_in_maps(pred, target, mu, logvar)
    nc = _build_program()
    res = run_bass_kernel_spmd(nc, in_maps, list(range(CORES)))
    return _host_post(res.results)


# revision 31
# speedup vs baseline: 1.0348x; 1.0348x over previous
"""Trainium2 Bass kernel for MeshGenLoss (Chamfer + KL + density-uniformity).

Math: d[i,j] = |a_i|^2 + |b_j|^2 - 2 a_i.b_j as ONE K=33 bf16 matmul per
[128,512] tile (3 exact bf16 limbs per fp32 scalar -> fp32-exact distances
in PSUM at bf16 matmul speed).

v4 structure ("negated world" + banded pp):
 - Everything evacuated from PSUM is NEGATED (ACT copy scale=-1, bf16), so
   all min-reductions become MAX ops; host negates at the end.
 - pt (pred x target): full [512 x 4096] rows per core. Row-maxes (pred-side
   Chamfer) via DVE bf16 TT-fold chains; column-maxes (target-side) via
   slab max-merge -> DMA transpose -> 3D fold -> reduce.
 - pp (pred self-distance) uses SYMMETRY HARDER than v3: each 128-row block
   g computes only col-blocks (g+k) mod 32 for k in 0..16 (17 blocks = 2176
   cols instead of 4096). Every unordered pair lands in the band of one of
   its two blocks, so nn_pp = min(band row-mins, band col-mins). This cuts
   pp matmul + evacuation + merge volume by ~47% at the cost of adding pp
   row-min folds (cheap, bf16 2x).
 - pp slab merges run on GpSimd (idle engine) to unload DVE.
 - Diagonal mask: (1000*I)^T@(1000*I) accumulated into the first 128 cols
   of each pp row-block's chunk0 (band starts at the diagonal).

Sharding: core c owns pred/target rows [512c, 512c+512). rhs_p columns are
host-rotated by -512c and sliced to [0,2560) so each row-block r's band is
the contiguous rotated cols [128r, 128r+2176) (identical SPMD program).
"""

import sys

import ml_dtypes
import numpy as np

sys.path.insert(0, "/opt/trn_rl_repo")

B = 2
N = 4096
L = 512
CORES = 8
ROWS = N // CORES  # 512 rows per core
RB = ROWS // 128  # 4 row blocks per core
K = 33
BF16 = ml_dtypes.bfloat16
NBLK = N // 128  # 32 global 128-blocks
BANDB = 17  # col blocks per pp row-block band
BANDW = BANDB * 128  # 2176
PPW = (RB - 1) * 128 + BANDW  # 2560 rotated pp cols touched per core
PPB = PPW // 128  # 20

NEG_BIG = -3.0e38

# per-batch job order: seeds first, pt slab completes early (transpose
# overlaps pp2/pp3), pp slab last (shorter finish chain).
JOB_ORDER = [("pt", 0), ("pp", 0), ("pt", 1), ("pp", 1),
             ("pt", 2), ("pt", 3), ("pp", 2), ("pp", 3)]


def _limbs3(x):
    """Split float64 array into 3 bf16 limbs capturing ~24 significand bits."""
    h = x.astype(BF16)
    r = x - h.astype(np.float64)
    m = r.astype(BF16)
    r2 = r - m.astype(np.float64)
    lo = r2.astype(BF16)
    return h, m, lo


def _build_lhsT(a):
    """a: [n, 3] float64 row points -> lhsT [33, n] bf16."""
    n = a.shape[0]
    asq = (a * a).sum(-1)
    al = _limbs3(a)
    sl = _limbs3(asq)
    out = np.zeros((K, n), dtype=BF16)
    k = 0
    for t in range(3):
        for p in range(3):
            row = (-2.0 * al[p][:, t].astype(np.float64)).astype(BF16)
            for _q in range(3):
                out[k] = row
                k += 1
    for p in range(3):
        out[k] = sl[p]
        k += 1
    for _q in range(3):
        out[k] = np.ones(n, dtype=BF16)
        k += 1
    return out


def _build_rhs(b):
    """b: [m, 3] float64 column points -> rhs [33, m] bf16."""
    m = b.shape[0]
    bsq = (b * b).sum(-1)
    bl = _limbs3(b)
    sl = _limbs3(bsq)
    out = np.zeros((K, m), dtype=BF16)
    k = 0
    for t in range(3):
        for _p in range(3):
            for q in range(3):
                out[k] = bl[q][:, t]
                k += 1
    for _p in range(3):
        out[k] = np.ones(m, dtype=BF16)
        k += 1
    for q in range(3):
        out[k] = sl[q]
        k += 1
    return out


def _build_program():
    import concourse.bacc as bacc
    import concourse.mybir as mybir
    import concourse.tile as tile
    from contextlib import ExitStack

    dt = mybir.dt
    Alu = mybir.AluOpType
    Act = mybir.ActivationFunctionType

    nc = bacc.Bacc("TRN2", target_bir_lowering=False, debug=False)

    d_lhsT = nc.declare_dram_parameter("lhsT", [B, K, ROWS], dt.bfloat16, isOutput=False)
    d_rhs_t = nc.declare_dram_parameter("rhs_t", [B, K, N], dt.bfloat16, isOutput=False)
    d_rhs_p = nc.declare_dram_parameter("rhs_p", [B, K, PPW], dt.bfloat16, isOutput=False)
    d_dql = nc.declare_dram_parameter("dql", [128, 128], dt.bfloat16, isOutput=False)
    d_mu = nc.declare_dram_parameter("mu_sl", [1, 128], dt.float32, isOutput=False)
    d_lv = nc.declare_dram_parameter("lv_sl", [1, 128], dt.float32, isOutput=False)

    # o_min: NEGATED row maxes; col 4b+r = pt, col 8+4b+r = pp band
    o_min = nc.declare_dram_parameter("o_min", [128, 16], dt.float32, isOutput=True)
    # o_ct: NEGATED col maxes: pt b: cols 32b..32b+32 (block t);
    #       pp b: cols 64+20b..64+20b+20 (rotated block p)
    o_ct = nc.declare_dram_parameter("o_ct", [128, 104], dt.float32, isOutput=True)
    o_kl = nc.declare_dram_parameter("o_kl", [1, 3], dt.float32, isOutput=True)

    with tile.TileContext(nc) as tc, ExitStack() as ctx:
        consts = ctx.enter_context(tc.tile_pool(name="consts", bufs=1))
        psum = ctx.enter_context(tc.tile_pool(name="psum", bufs=2, space="PSUM"))
        stpool = ctx.enter_context(tc.tile_pool(name="st", bufs=4))
        utpool = ctx.enter_context(tc.tile_pool(name="ut", bufs=2))
        fpool = ctx.enter_context(tc.tile_pool(name="f", bufs=2))

        # ---- resident inputs --------------------------------------------
        lhsT_sb = {}
        rhs_sb = {}
        for b in range(B):
            t1 = consts.tile([K, ROWS], dt.bfloat16, tag=f"l{b}")
            rt = consts.tile([K, N], dt.bfloat16, tag=f"rt{b}")
            rp = consts.tile([K, PPW], dt.bfloat16, tag=f"rp{b}")
            lhsT_sb[b] = t1
            rhs_sb["pt", b] = rt
            rhs_sb["pp", b] = rp
        dql_sb = consts.tile([128, 128], dt.bfloat16, tag="dql")
        mu_sb = consts.tile([1, 128], dt.float32, tag="mu")
        lv_sb = consts.tile([1, 128], dt.float32, tag="lv")

        # input DMAs: the first matmul needs lhsT[0] + rhs_t[0][:, :512] —
        # issue those in PARALLEL on different queues; bulk follows.
        nc.sync.dma_start(out=lhsT_sb[0][:], in_=d_lhsT[0])
        nc.gpsimd.dma_start(out=rhs_sb["pt", 0][:, :512], in_=d_rhs_t[0, :, :512])
        nc.gpsimd.dma_start(out=dql_sb[:], in_=d_dql[:])
        nc.gpsimd.dma_start(out=rhs_sb["pt", 0][:, 512:2048], in_=d_rhs_t[0, :, 512:2048])
        nc.sync.dma_start(out=rhs_sb["pt", 0][:, 2048:], in_=d_rhs_t[0, :, 2048:])
        nc.gpsimd.dma_start(out=rhs_sb["pp", 0][:, :1280], in_=d_rhs_p[0, :, :1280])
        nc.sync.dma_start(out=rhs_sb["pp", 0][:, 1280:], in_=d_rhs_p[0, :, 1280:])
        nc.sync.dma_start(out=lhsT_sb[1][:], in_=d_lhsT[1])
        nc.gpsimd.dma_start(out=rhs_sb["pt", 1][:, :2048], in_=d_rhs_t[1, :, :2048])
        nc.sync.dma_start(out=rhs_sb["pt", 1][:, 2048:], in_=d_rhs_t[1, :, 2048:])
        nc.gpsimd.dma_start(out=rhs_sb["pp", 1][:], in_=d_rhs_p[1])
        nc.gpsimd.dma_start(out=mu_sb[:], in_=d_mu[:])
        nc.gpsimd.dma_start(out=lv_sb[:], in_=d_lv[:])

        omin_sb = consts.tile([128, 16], dt.float32, tag="omin")
        ctall = consts.tile([128, 104], dt.float32, tag="ctall")

        # persistent slabs (negated, max-merged)
        slab = {}
        for b in range(B):
            spt = consts.tile([128, N], dt.bfloat16, tag=f"spt{b}")
            spp = consts.tile([128, PPW], dt.bfloat16, tag=f"spp{b}")
            slab["pt", b] = spt
            slab["pp", b] = spp

        def finish_slab(kind, b):
            """slab -> DMA transpose -> 3D TT-fold -> reduce into ctall."""
            if kind == "pt":
                nb, off = NBLK, 32 * b
            else:
                nb, off = PPB, 64 + PPB * b
            ut = utpool.tile([128, nb, 128], dt.bfloat16, tag=f"ut{kind}")
            nc.sync.dma_start_transpose(ut[:], slab[kind, b][:])
            g1 = fpool.tile([128, nb, 64], dt.bfloat16, tag=f"g1{kind}")
            nc.vector.tensor_tensor(g1[:], ut[:, :, 0:64], ut[:, :, 64:128], Alu.max)
            g2 = fpool.tile([128, nb, 32], dt.bfloat16, tag=f"g2{kind}")
            nc.vector.tensor_tensor(g2[:], g1[:, :, 0:32], g1[:, :, 32:64], Alu.max)
            g3 = fpool.tile([128, nb, 16], dt.bfloat16, tag=f"g3{kind}")
            nc.vector.tensor_tensor(g3[:], g2[:, :, 0:16], g2[:, :, 16:32], Alu.max)
            nc.vector.tensor_reduce(
                ctall[:, off:off + nb], g3[:], axis=mybir.AxisListType.X, op=Alu.max)

        for b in range(B):
            for kind, r in JOB_ORDER:
                lhsT = lhsT_sb[b][:, 128 * r:128 * (r + 1)]
                rhs = rhs_sb[kind, b]
                if kind == "pt":
                    base, w1 = 0, 2048  # second chunk width
                else:
                    base, w1 = 128 * r, 128
                if r == 0:
                    st = slab[kind, b]
                else:
                    st = stpool.tile([128, N if kind == "pt" else BANDW],
                                     dt.bfloat16, tag=f"st{kind}")
                # ---- chunk 0: [128, 2048] = 4 matmuls (+ diag for pp) ----
                ch0 = psum.tile([128, 2048], dt.float32, tag="ps")
                if kind == "pp":
                    # diag block [0:128] is its own accumulation group
                    nc.tensor.matmul(
                        ch0[:, 0:128], lhsT, rhs[:, base:base + 128],
                        start=True, stop=False)
                    nc.tensor.matmul(
                        ch0[:, 0:128], dql_sb[:], dql_sb[:],
                        start=False, stop=True)
                    nc.tensor.matmul(
                        ch0[:, 128:512], lhsT, rhs[:, base + 128:base + 512],
                        start=True, stop=True)
                    t0 = 1
                else:
                    t0 = 0
                for t in range(t0, 4):
                    c0 = base + 512 * t
                    nc.tensor.matmul(
                        ch0[:, 512 * t:512 * (t + 1)], lhsT, rhs[:, c0:c0 + 512],
                        start=True, stop=True)
                nc.scalar.activation(st[:, 0:2048], ch0[:], Act.Copy, scale=-1.0)
                # ---- chunk 1: pt [128,2048], pp [128,128] ----------------
                ch1 = psum.tile([128, 2048], dt.float32, tag="ps")
                nmm = w1 // 512 if w1 >= 512 else 1
                for t in range(nmm):
                    c0 = base + 2048 + 512 * t
                    cw = min(512, w1 - 512 * t)
                    nc.tensor.matmul(
                        ch1[:, 512 * t:512 * t + cw], lhsT, rhs[:, c0:c0 + cw],
                        start=True, stop=True)
                nc.scalar.activation(
                    st[:, 2048:2048 + w1], ch1[:, 0:w1], Act.Copy, scale=-1.0)
                # ---- row-max fold chain (TT bf16 2x) + final reduce ------
                col = 4 * b + r if kind == "pt" else 8 + 4 * b + r
                if kind == "pt":
                    f1 = fpool.tile([128, 2048], dt.bfloat16, tag="f1")
                    nc.vector.tensor_tensor(
                        f1[:], st[:, 0:2048], st[:, 2048:4096], Alu.max)
                    f2 = fpool.tile([128, 1024], dt.bfloat16, tag="f2")
                    nc.vector.tensor_tensor(
                        f2[:], f1[:, 0:1024], f1[:, 1024:2048], Alu.max)
                    f3 = fpool.tile([128, 512], dt.bfloat16, tag="f3")
                    nc.vector.tensor_tensor(
                        f3[:], f2[:, 0:512], f2[:, 512:1024], Alu.max)
                    nc.vector.tensor_reduce(
                        omin_sb[:, col:col + 1], f3[:],
                        axis=mybir.AxisListType.X, op=Alu.max)
                else:
                    q1 = fpool.tile([128, 1088], dt.bfloat16, tag="q1")
                    nc.vector.tensor_tensor(
                        q1[:], st[:, 0:1088], st[:, 1088:2176], Alu.max)
                    q2 = fpool.tile([128, 544], dt.bfloat16, tag="q2")
                    nc.vector.tensor_tensor(
                        q2[:], q1[:, 0:544], q1[:, 544:1088], Alu.max)
                    nc.vector.tensor_reduce(
                        omin_sb[:, col:col + 1], q2[:],
                        axis=mybir.AxisListType.X, op=Alu.max)
                # ---- slab merge ------------------------------------------
                if kind == "pp" and r == 0:
                    # init the slab tail the r=0 band doesn't reach
                    nc.vector.memset(slab["pp", b][:, BANDW:PPW], NEG_BIG)
                if r > 0:
                    if kind == "pt":
                        nc.vector.tensor_tensor(
                            slab["pt", b][:], slab["pt", b][:], st[:], Alu.max)
                    else:
                        win = slab["pp", b][:, base:base + BANDW]
                        nc.vector.tensor_tensor(win, win, st[:], Alu.max)
                if (kind, r) in (("pt", 3), ("pp", 3)):
                    finish_slab(kind, b)
        # ---- KL partials (at the end: avoids ACT table switches mid-
        # stream; Exp/Square live in a different table set than Copy) ----
        klt = consts.tile([1, 3], dt.float32, tag="klt")
        cp_t = consts.tile([1, 128], dt.float32, tag="klcp")
        nc.scalar.activation(cp_t[:], lv_sb[:], Act.Copy, accum_out=klt[:, 0:1])
        e_t = consts.tile([1, 128], dt.float32, tag="klexp")
        nc.scalar.activation(e_t[:], lv_sb[:], Act.Exp, accum_out=klt[:, 2:3])
        sq_t = consts.tile([1, 128], dt.float32, tag="klsq")
        nc.scalar.activation(sq_t[:], mu_sb[:], Act.Square, accum_out=klt[:, 1:2])

        # ---- outputs ----------------------------------------------------
        nc.sync.dma_start(out=o_min[:], in_=omin_sb[:])
        nc.sync.dma_start(out=o_ct[:], in_=ctall[:])
        nc.sync.dma_start(out=o_kl[:], in_=klt[:])

    nc.compile()
    return nc


def _make_in_maps(pred, target, mu, logvar):
    pred = np.asarray(pred, dtype=np.float32)
    target = np.asarray(target, dtype=np.float32)
    mu = np.asarray(mu, dtype=np.float32)
    logvar = np.asarray(logvar, dtype=np.float32)

    pred64 = pred.astype(np.float64)
    target64 = target.astype(np.float64)

    rhs_t = np.stack([_build_rhs(target64[b]) for b in range(B)])  # [B,K,N]
    rhs_p_full = np.stack([_build_rhs(pred64[b]) for b in range(B)])
    dql = (np.eye(128) * 1000.0).astype(BF16)
    mu_flat = mu.reshape(-1)
    lv_flat = logvar.reshape(-1)

    in_maps = []
    for c in range(CORES):
        rows = slice(ROWS * c, ROWS * (c + 1))
        lhsT = np.stack([_build_lhsT(pred64[b, rows]) for b in range(B)])
        rot = np.roll(rhs_p_full, -ROWS * c, axis=2)[:, :, :PPW]
        in_maps.append({
            "lhsT": lhsT,
            "rhs_t": rhs_t,
            "rhs_p": np.ascontiguousarray(rot),
            "dql": dql,
            "mu_sl": mu_flat[128 * c:128 * (c + 1)].reshape(1, 128),
            "lv_sl": lv_flat[128 * c:128 * (c + 1)].reshape(1, 128),
        })
    return in_maps


def _host_post(results):
    # row mins (negated maxes): pt cols 0..7, pp cols 8..15
    nn_pt = np.empty((B, N), dtype=np.float64)
    pp_row = np.empty((B, N), dtype=np.float64)
    for c in range(CORES):
        om = -results[c]["o_min"].astype(np.float64)  # [128, 16]
        for b in range(B):
            for r in range(RB):
                rows = slice(ROWS * c + 128 * r, ROWS * c + 128 * r + 128)
                nn_pt[b, rows] = om[:, 4 * b + r]
                pp_row[b, rows] = om[:, 8 + 4 * b + r]

    # col mins: o_ct [128, 104] negated
    cts = -np.stack([r["o_ct"] for r in results]).astype(np.float64)
    nn_tp = np.full((B, N), np.inf)
    pp_col = np.full((B, N), np.inf)
    for c in range(CORES):
        for b in range(B):
            pt_v = cts[c][:, 32 * b:32 * b + 32]  # [jrel, t]
            nn_tp[b] = np.minimum(nn_tp[b], pt_v.T.reshape(N))
            pp_v = cts[c][:, 64 + PPB * b:64 + PPB * b + PPB]  # [jrel, p]
            for p in range(PPB):
                gblk = (4 * c + p) % NBLK
                cols = slice(128 * gblk, 128 * gblk + 128)
                pp_col[b, cols] = np.minimum(pp_col[b, cols], pp_v[:, p])
    nn_pp = np.minimum(pp_row, pp_col)

    kl_parts = np.stack([r["o_kl"].reshape(3) for r in results])

    cd = (nn_pt.mean(axis=1) + nn_tp.mean(axis=1)).mean()

    s1 = kl_parts[:, 0].astype(np.float64).sum()
    s2 = kl_parts[:, 1].astype(np.float64).sum()
    s3 = kl_parts[:, 2].astype(np.float64).sum()
    n_kl = B * L
    kl = -0.5 * (n_kl + s1 - s2 - s3) / n_kl

    density = np.std(nn_pp, axis=1, ddof=1).mean()

    total = cd + 0.001 * kl + 0.1 * density

    return (
        np.float32(total),
        np.float32(cd),
        np.float32(kl),
        np.float32(density),
    )


def kernel(pred, target, mu, logvar):
    from concourse.bass_utils import run_bass_kernel_spmd

    in_maps = _make_in_maps(pred, target, mu, logvar)
    nc = _build_program()
    res = run_bass_kernel_spmd(nc, in_maps, list(range(CORES)))
    return _host_post(res.results)


# revision 35
# speedup vs baseline: 1.0392x; 1.0042x over previous
"""Trainium2 Bass kernel for MeshGenLoss (Chamfer + KL + density-uniformity).

Math: d[i,j] = |a_i|^2 + |b_j|^2 - 2 a_i.b_j as ONE K=33 bf16 matmul per
[128,512] tile (3 exact bf16 limbs per fp32 scalar -> fp32-exact distances
in PSUM at bf16 matmul speed).

v4 structure ("negated world" + banded pp):
 - Everything evacuated from PSUM is NEGATED (ACT copy scale=-1, bf16), so
   all min-reductions become MAX ops; host negates at the end.
 - pt (pred x target): full [512 x 4096] rows per core. Row-maxes (pred-side
   Chamfer) via DVE bf16 TT-fold chains; column-maxes (target-side) via
   slab max-merge -> DMA transpose -> 3D fold -> reduce.
 - pp (pred self-distance) uses SYMMETRY HARDER than v3: each 128-row block
   g computes only col-blocks (g+k) mod 32 for k in 0..16 (17 blocks = 2176
   cols instead of 4096). Every unordered pair lands in the band of one of
   its two blocks, so nn_pp = min(band row-mins, band col-mins). This cuts
   pp matmul + evacuation + merge volume by ~47% at the cost of adding pp
   row-min folds (cheap, bf16 2x).
 - pp slab merges run on GpSimd (idle engine) to unload DVE.
 - Diagonal mask: (1000*I)^T@(1000*I) accumulated into the first 128 cols
   of each pp row-block's chunk0 (band starts at the diagonal).

Sharding: core c owns pred/target rows [512c, 512c+512). rhs_p columns are
host-rotated by -512c and sliced to [0,2560) so each row-block r's band is
the contiguous rotated cols [128r, 128r+2176) (identical SPMD program).
"""

import sys

import ml_dtypes
import numpy as np

sys.path.insert(0, "/opt/trn_rl_repo")

B = 2
N = 4096
L = 512
CORES = 8
ROWS = N // CORES  # 512 rows per core
RB = ROWS // 128  # 4 row blocks per core
K = 33
BF16 = ml_dtypes.bfloat16
NBLK = N // 128  # 32 global 128-blocks
BANDB = 17  # col blocks per pp row-block band
BANDW = BANDB * 128  # 2176
PPW = (RB - 1) * 128 + BANDW  # 2560 rotated pp cols touched per core
PPB = PPW // 128  # 20

NEG_BIG = -3.0e38

# per-batch job order: seeds first, pt slab completes early (transpose
# overlaps pp2/pp3), pp slab last (shorter finish chain).
JOB_ORDER = [("pt", 0), ("pp", 0), ("pt", 1), ("pp", 1),
             ("pt", 2), ("pt", 3), ("pp", 2), ("pp", 3)]


def _limbs3(x):
    """Split float64 array into 3 bf16 limbs capturing ~24 significand bits."""
    h = x.astype(BF16)
    r = x - h.astype(np.float64)
    m = r.astype(BF16)
    r2 = r - m.astype(np.float64)
    lo = r2.astype(BF16)
    return h, m, lo


def _build_lhsT(a):
    """a: [n, 3] float64 row points -> lhsT [33, n] bf16."""
    n = a.shape[0]
    asq = (a * a).sum(-1)
    al = _limbs3(a)
    sl = _limbs3(asq)
    out = np.zeros((K, n), dtype=BF16)
    k = 0
    for t in range(3):
        for p in range(3):
            row = (-2.0 * al[p][:, t].astype(np.float64)).astype(BF16)
            for _q in range(3):
                out[k] = row
                k += 1
    for p in range(3):
        out[k] = sl[p]
        k += 1
    for _q in range(3):
        out[k] = np.ones(n, dtype=BF16)
        k += 1
    return out


def _build_rhs(b):
    """b: [m, 3] float64 column points -> rhs [33, m] bf16."""
    m = b.shape[0]
    bsq = (b * b).sum(-1)
    bl = _limbs3(b)
    sl = _limbs3(bsq)
    out = np.zeros((K, m), dtype=BF16)
    k = 0
    for t in range(3):
        for _p in range(3):
            for q in range(3):
                out[k] = bl[q][:, t]
                k += 1
    for _p in range(3):
        out[k] = np.ones(m, dtype=BF16)
        k += 1
    for q in range(3):
        out[k] = sl[q]
        k += 1
    return out


def _build_program():
    import concourse.bacc as bacc
    import concourse.mybir as mybir
    import concourse.tile as tile
    from contextlib import ExitStack

    dt = mybir.dt
    Alu = mybir.AluOpType
    Act = mybir.ActivationFunctionType

    nc = bacc.Bacc("TRN2", target_bir_lowering=False, debug=False)

    d_lhsT = nc.declare_dram_parameter("lhsT", [B, K, ROWS], dt.bfloat16, isOutput=False)
    d_rhs_t = nc.declare_dram_parameter("rhs_t", [B, K, N], dt.bfloat16, isOutput=False)
    d_rhs_p = nc.declare_dram_parameter("rhs_p", [B, K, PPW], dt.bfloat16, isOutput=False)
    d_dql = nc.declare_dram_parameter("dql", [128, 128], dt.bfloat16, isOutput=False)
    d_mu = nc.declare_dram_parameter("mu_sl", [1, 128], dt.float32, isOutput=False)
    d_lv = nc.declare_dram_parameter("lv_sl", [1, 128], dt.float32, isOutput=False)

    # o_min: NEGATED row maxes; col 4b+r = pt, col 8+4b+r = pp band
    o_min = nc.declare_dram_parameter("o_min", [128, 16], dt.float32, isOutput=True)
    # o_ct: NEGATED col maxes: pt b: cols 32b..32b+32 (block t);
    #       pp b: cols 64+20b..64+20b+20 (rotated block p)
    o_ct = nc.declare_dram_parameter("o_ct", [128, 104], dt.float32, isOutput=True)
    o_kl = nc.declare_dram_parameter("o_kl", [1, 3], dt.float32, isOutput=True)

    with tile.TileContext(nc) as tc, ExitStack() as ctx:
        consts = ctx.enter_context(tc.tile_pool(name="consts", bufs=1))
        psum = ctx.enter_context(tc.tile_pool(name="psum", bufs=2, space="PSUM"))
        stpool = ctx.enter_context(tc.tile_pool(name="st", bufs=4))
        utpool = ctx.enter_context(tc.tile_pool(name="ut", bufs=2))
        fpool = ctx.enter_context(tc.tile_pool(name="f", bufs=2))

        # ---- resident inputs --------------------------------------------
        lhsT_sb = {}
        rhs_sb = {}
        for b in range(B):
            t1 = consts.tile([K, ROWS], dt.bfloat16, tag=f"l{b}")
            rt = consts.tile([K, N], dt.bfloat16, tag=f"rt{b}")
            rp = consts.tile([K, PPW], dt.bfloat16, tag=f"rp{b}")
            lhsT_sb[b] = t1
            rhs_sb["pt", b] = rt
            rhs_sb["pp", b] = rp
        dql_sb = consts.tile([128, 128], dt.bfloat16, tag="dql")
        mu_sb = consts.tile([1, 128], dt.float32, tag="mu")
        lv_sb = consts.tile([1, 128], dt.float32, tag="lv")

        # input DMAs: the first matmul needs lhsT[0] + rhs_t[0][:, :512] —
        # issue those in PARALLEL on different queues; bulk follows.
        nc.sync.dma_start(out=lhsT_sb[0][:], in_=d_lhsT[0])
        nc.gpsimd.dma_start(out=rhs_sb["pt", 0][:, :512], in_=d_rhs_t[0, :, :512])
        nc.gpsimd.dma_start(out=dql_sb[:], in_=d_dql[:])
        nc.gpsimd.dma_start(out=rhs_sb["pt", 0][:, 512:2048], in_=d_rhs_t[0, :, 512:2048])
        nc.sync.dma_start(out=rhs_sb["pt", 0][:, 2048:], in_=d_rhs_t[0, :, 2048:])
        nc.gpsimd.dma_start(out=rhs_sb["pp", 0][:, :1280], in_=d_rhs_p[0, :, :1280])
        nc.sync.dma_start(out=rhs_sb["pp", 0][:, 1280:], in_=d_rhs_p[0, :, 1280:])
        nc.sync.dma_start(out=lhsT_sb[1][:], in_=d_lhsT[1])
        nc.gpsimd.dma_start(out=rhs_sb["pt", 1][:, :2048], in_=d_rhs_t[1, :, :2048])
        nc.sync.dma_start(out=rhs_sb["pt", 1][:, 2048:], in_=d_rhs_t[1, :, 2048:])
        nc.gpsimd.dma_start(out=rhs_sb["pp", 1][:], in_=d_rhs_p[1])
        nc.gpsimd.dma_start(out=mu_sb[:], in_=d_mu[:])
        nc.gpsimd.dma_start(out=lv_sb[:], in_=d_lv[:])

        omin_sb = consts.tile([128, 16], dt.float32, tag="omin")
        ctall = consts.tile([128, 104], dt.float32, tag="ctall")

        # persistent slabs (negated, max-merged)
        slab = {}
        for b in range(B):
            spt = consts.tile([128, N], dt.bfloat16, tag=f"spt{b}")
            spp = consts.tile([128, PPW], dt.bfloat16, tag=f"spp{b}")
            slab["pt", b] = spt
            slab["pp", b] = spp

        def start_finish(kind, b):
            """Issue the slab transpose DMA; DVE folds come later (deferred
            past the next job so DVE isn't head-of-line blocked on the DMA)."""
            nb = NBLK if kind == "pt" else PPB
            ut = utpool.tile([128, nb, 128], dt.bfloat16, tag=f"ut{kind}")
            nc.sync.dma_start_transpose(ut[:], slab[kind, b][:])
            return ut

        def end_finish(kind, b, ut):
            if kind == "pt":
                nb, off = NBLK, 32 * b
            else:
                nb, off = PPB, 64 + PPB * b
            g1 = fpool.tile([128, nb, 64], dt.bfloat16, tag=f"g1{kind}")
            nc.vector.tensor_tensor(g1[:], ut[:, :, 0:64], ut[:, :, 64:128], Alu.max)
            g2 = fpool.tile([128, nb, 32], dt.bfloat16, tag=f"g2{kind}")
            nc.vector.tensor_tensor(g2[:], g1[:, :, 0:32], g1[:, :, 32:64], Alu.max)
            g3 = fpool.tile([128, nb, 16], dt.bfloat16, tag=f"g3{kind}")
            nc.vector.tensor_tensor(g3[:], g2[:, :, 0:16], g2[:, :, 16:32], Alu.max)
            nc.vector.tensor_reduce(
                ctall[:, off:off + nb], g3[:], axis=mybir.AxisListType.X, op=Alu.max)

        pending = []  # deferred finish chains: (kind, b, ut)
        for b in range(B):
            for kind, r in JOB_ORDER:
                while pending:
                    end_finish(*pending.pop(0))
                lhsT = lhsT_sb[b][:, 128 * r:128 * (r + 1)]
                rhs = rhs_sb[kind, b]
                if kind == "pt":
                    base, w1 = 0, 2048  # second chunk width
                else:
                    base, w1 = 128 * r, 128
                if r == 0:
                    st = slab[kind, b]
                else:
                    st = stpool.tile([128, N if kind == "pt" else BANDW],
                                     dt.bfloat16, tag=f"st{kind}")
                # ---- chunk 0: [128, 2048] = 4 matmuls (+ diag for pp) ----
                ch0 = psum.tile([128, 2048], dt.float32, tag="ps")
                if kind == "pp":
                    # diag block [0:128] is its own accumulation group
                    nc.tensor.matmul(
                        ch0[:, 0:128], lhsT, rhs[:, base:base + 128],
                        start=True, stop=False)
                    nc.tensor.matmul(
                        ch0[:, 0:128], dql_sb[:], dql_sb[:],
                        start=False, stop=True)
                    nc.tensor.matmul(
                        ch0[:, 128:512], lhsT, rhs[:, base + 128:base + 512],
                        start=True, stop=True)
                    t0 = 1
                else:
                    t0 = 0
                for t in range(t0, 4):
                    c0 = base + 512 * t
                    nc.tensor.matmul(
                        ch0[:, 512 * t:512 * (t + 1)], lhsT, rhs[:, c0:c0 + 512],
                        start=True, stop=True)
                nc.scalar.activation(st[:, 0:2048], ch0[:], Act.Copy, scale=-1.0)
                # ---- chunk 1: pt [128,2048], pp [128,128] ----------------
                ch1 = psum.tile([128, 2048], dt.float32, tag="ps")
                nmm = w1 // 512 if w1 >= 512 else 1
                for t in range(nmm):
                    c0 = base + 2048 + 512 * t
                    cw = min(512, w1 - 512 * t)
                    nc.tensor.matmul(
                        ch1[:, 512 * t:512 * t + cw], lhsT, rhs[:, c0:c0 + cw],
                        start=True, stop=True)
                nc.scalar.activation(
                    st[:, 2048:2048 + w1], ch1[:, 0:w1], Act.Copy, scale=-1.0)
                # ---- row-max fold chain (TT bf16 2x) + final reduce ------
                col = 4 * b + r if kind == "pt" else 8 + 4 * b + r
                if kind == "pt":
                    f1 = fpool.tile([128, 2048], dt.bfloat16, tag="f1")
                    nc.vector.tensor_tensor(
                        f1[:], st[:, 0:2048], st[:, 2048:4096], Alu.max)
                    f2 = fpool.tile([128, 1024], dt.bfloat16, tag="f2")
                    nc.vector.tensor_tensor(
                        f2[:], f1[:, 0:1024], f1[:, 1024:2048], Alu.max)
                    f3 = fpool.tile([128, 512], dt.bfloat16, tag="f3")
                    nc.vector.tensor_tensor(
                        f3[:], f2[:, 0:512], f2[:, 512:1024], Alu.max)
                    nc.vector.tensor_reduce(
                        omin_sb[:, col:col + 1], f3[:],
                        axis=mybir.AxisListType.X, op=Alu.max)
                else:
                    q1 = fpool.tile([128, 1088], dt.bfloat16, tag="q1")
                    nc.vector.tensor_tensor(
                        q1[:], st[:, 0:1088], st[:, 1088:2176], Alu.max)
                    q2 = fpool.tile([128, 544], dt.bfloat16, tag="q2")
                    nc.vector.tensor_tensor(
                        q2[:], q1[:, 0:544], q1[:, 544:1088], Alu.max)
                    nc.vector.tensor_reduce(
                        omin_sb[:, col:col + 1], q2[:],
                        axis=mybir.AxisListType.X, op=Alu.max)
                # ---- slab merge ------------------------------------------
                if kind == "pp" and r == 0:
                    # init the slab tail the r=0 band doesn't reach
                    nc.vector.memset(slab["pp", b][:, BANDW:PPW], NEG_BIG)
                if r > 0:
                    if kind == "pt":
                        nc.vector.tensor_tensor(
                            slab["pt", b][:], slab["pt", b][:], st[:], Alu.max)
                    else:
                        win = slab["pp", b][:, base:base + BANDW]
                        nc.vector.tensor_tensor(win, win, st[:], Alu.max)
                if (kind, r) in (("pt", 3), ("pp", 3)):
                    pending.append((kind, b, start_finish(kind, b)))
        while pending:
            end_finish(*pending.pop(0))
        # ---- KL partials (at the end: avoids ACT table switches mid-
        # stream; Exp/Square live in a different table set than Copy) ----
        klt = consts.tile([1, 3], dt.float32, tag="klt")
        cp_t = consts.tile([1, 128], dt.float32, tag="klcp")
        nc.scalar.activation(cp_t[:], lv_sb[:], Act.Copy, accum_out=klt[:, 0:1])
        e_t = consts.tile([1, 128], dt.float32, tag="klexp")
        nc.scalar.activation(e_t[:], lv_sb[:], Act.Exp, accum_out=klt[:, 2:3])
        sq_t = consts.tile([1, 128], dt.float32, tag="klsq")
        nc.scalar.activation(sq_t[:], mu_sb[:], Act.Square, accum_out=klt[:, 1:2])

        # ---- outputs ----------------------------------------------------
        nc.sync.dma_start(out=o_min[:], in_=omin_sb[:])
        nc.sync.dma_start(out=o_ct[:], in_=ctall[:])
        nc.sync.dma_start(out=o_kl[:], in_=klt[:])

    nc.compile()
    return nc


def _make_in_maps(pred, target, mu, logvar):
    pred = np.asarray(pred, dtype=np.float32)
    target = np.asarray(target, dtype=np.float32)
    mu = np.asarray(mu, dtype=np.float32)
    logvar = np.asarray(logvar, dtype=np.float32)

    pred64 = pred.astype(np.float64)
    target64 = target.astype(np.float64)

    rhs_t = np.stack([_build_rhs(target64[b]) for b in range(B)])  # [B,K,N]
    rhs_p_full = np.stack([_build_rhs(pred64[b]) for b in range(B)])
    dql = (np.eye(128) * 1000.0).astype(BF16)
    mu_flat = mu.reshape(-1)
    lv_flat = logvar.reshape(-1)

    in_maps = []
    for c in range(CORES):
        rows = slice(ROWS * c, ROWS * (c + 1))
        lhsT = np.stack([_build_lhsT(pred64[b, rows]) for b in range(B)])
        rot = np.roll(rhs_p_full, -ROWS * c, axis=2)[:, :, :PPW]
        in_maps.append({
            "lhsT": lhsT,
            "rhs_t": rhs_t,
            "rhs_p": np.ascontiguousarray(rot),
            "dql": dql,
            "mu_sl": mu_flat[128 * c:128 * (c + 1)].reshape(1, 128),
            "lv_sl": lv_flat[128 * c:128 * (c + 1)].reshape(1, 128),
        })
    return in_maps


def _host_post(results):
    # row mins (negated maxes): pt cols 0..7, pp cols 8..15
    nn_pt = np.empty((B, N), dtype=np.float64)
    pp_row = np.empty((B, N), dtype=np.float64)
    for c in range(CORES):
        om = -results[c]["o_min"].astype(np.float64)  # [128, 16]
        for b in range(B):
            for r in range(RB):
                rows = slice(ROWS * c + 128 * r, ROWS * c + 128 * r + 128)
                nn_pt[b, rows] = om[:, 4 * b + r]
                pp_row[b, rows] = om[:, 8 + 4 * b + r]

    # col mins: o_ct [128, 104] negated
    cts = -np.stack([r["o_ct"] for r in results]).astype(np.float64)
    nn_tp = np.full((B, N), np.inf)
    pp_col = np.full((B, N), np.inf)
    for c in range(CORES):
        for b in range(B):
            pt_v = cts[c][:, 32 * b:32 * b + 32]  # [jrel, t]
            nn_tp[b] = np.minimum(nn_tp[b], pt_v.T.reshape(N))
            pp_v = cts[c][:, 64 + PPB * b:64 + PPB * b + PPB]  # [jrel, p]
            for p in range(PPB):
                gblk = (4 * c + p) % NBLK
                cols = slice(128 * gblk, 128 * gblk + 128)
                pp_col[b, cols] = np.minimum(pp_col[b, cols], pp_v[:, p])
    nn_pp = np.minimum(pp_row, pp_col)

    kl_parts = np.stack([r["o_kl"].reshape(3) for r in results])

    cd = (nn_pt.mean(axis=1) + nn_tp.mean(axis=1)).mean()

    s1 = kl_parts[:, 0].astype(np.float64).sum()
    s2 = kl_parts[:, 1].astype(np.float64).sum()
    s3 = kl_parts[:, 2].astype(np.float64).sum()
    n_kl = B * L
    kl = -0.5 * (n_kl + s1 - s2 - s3) / n_kl

    density = np.std(nn_pp, axis=1, ddof=1).mean()

    total = cd + 0.001 * kl + 0.1 * density

    return (
        np.float32(total),
        np.float32(cd),
        np.float32(kl),
        np.float32(density),
    )


def kernel(pred, target, mu, logvar):
    from concourse.bass_utils import run_bass_kernel_spmd

    in_maps = _make_in_maps(pred, target, mu, logvar)
    nc = _build_program()
    res = run_bass_kernel_spmd(nc, in_maps, list(range(CORES)))
    return _host_post(res.results)


# revision 38
# speedup vs baseline: 1.0576x; 1.0177x over previous
"""Trainium2 Bass kernel for MeshGenLoss (Chamfer + KL + density-uniformity).

Math: d[i,j] = |a_i|^2 + |b_j|^2 - 2 a_i.b_j as ONE K=33 bf16 matmul per
[128,512] tile (3 exact bf16 limbs per fp32 scalar -> fp32-exact distances
in PSUM at bf16 matmul speed).

v4 structure ("negated world" + banded pp):
 - Everything evacuated from PSUM is NEGATED (ACT copy scale=-1, bf16), so
   all min-reductions become MAX ops; host negates at the end.
 - pt (pred x target): full [512 x 4096] rows per core. Row-maxes (pred-side
   Chamfer) via DVE bf16 TT-fold chains; column-maxes (target-side) via
   slab max-merge -> DMA transpose -> 3D fold -> reduce.
 - pp (pred self-distance) uses SYMMETRY HARDER than v3: each 128-row block
   g computes only col-blocks (g+k) mod 32 for k in 0..16 (17 blocks = 2176
   cols instead of 4096). Every unordered pair lands in the band of one of
   its two blocks, so nn_pp = min(band row-mins, band col-mins). This cuts
   pp matmul + evacuation + merge volume by ~47% at the cost of adding pp
   row-min folds (cheap, bf16 2x).
 - pp slab merges run on GpSimd (idle engine) to unload DVE.
 - Diagonal mask: (1000*I)^T@(1000*I) accumulated into the first 128 cols
   of each pp row-block's chunk0 (band starts at the diagonal).

Sharding: core c owns pred/target rows [512c, 512c+512). rhs_p columns are
host-rotated by -512c and sliced to [0,2560) so each row-block r's band is
the contiguous rotated cols [128r, 128r+2176) (identical SPMD program).
"""

import sys

import ml_dtypes
import numpy as np

sys.path.insert(0, "/opt/trn_rl_repo")

B = 2
N = 4096
L = 512
CORES = 8
ROWS = N // CORES  # 512 rows per core
RB = ROWS // 128  # 4 row blocks per core
K = 33
BF16 = ml_dtypes.bfloat16
NBLK = N // 128  # 32 global 128-blocks
BANDB = 17  # col blocks per pp row-block band
BANDW = BANDB * 128  # 2176
PPW = (RB - 1) * 128 + BANDW  # 2560 rotated pp cols touched per core
PPB = PPW // 128  # 20

NEG_BIG = -3.0e38

# per-batch job order: seeds first, pt slab completes early (transpose
# overlaps pp2/pp3), pp slab last (shorter finish chain).
JOB_ORDER = [("pt", 0), ("pp", 0), ("pt", 1), ("pp", 1),
             ("pt", 2), ("pt", 3), ("pp", 2), ("pp", 3)]


def _limbs3(x):
    """Split float64 array into 3 bf16 limbs capturing ~24 significand bits."""
    h = x.astype(BF16)
    r = x - h.astype(np.float64)
    m = r.astype(BF16)
    r2 = r - m.astype(np.float64)
    lo = r2.astype(BF16)
    return h, m, lo


def _build_lhsT(a):
    """a: [n, 3] float64 row points -> lhsT [33, n] bf16."""
    n = a.shape[0]
    asq = (a * a).sum(-1)
    al = _limbs3(a)
    sl = _limbs3(asq)
    out = np.zeros((K, n), dtype=BF16)
    k = 0
    for t in range(3):
        for p in range(3):
            row = (-2.0 * al[p][:, t].astype(np.float64)).astype(BF16)
            for _q in range(3):
                out[k] = row
                k += 1
    for p in range(3):
        out[k] = sl[p]
        k += 1
    for _q in range(3):
        out[k] = np.ones(n, dtype=BF16)
        k += 1
    return out


def _build_rhs(b):
    """b: [m, 3] float64 column points -> rhs [33, m] bf16."""
    m = b.shape[0]
    bsq = (b * b).sum(-1)
    bl = _limbs3(b)
    sl = _limbs3(bsq)
    out = np.zeros((K, m), dtype=BF16)
    k = 0
    for t in range(3):
        for _p in range(3):
            for q in range(3):
                out[k] = bl[q][:, t]
                k += 1
    for _p in range(3):
        out[k] = np.ones(m, dtype=BF16)
        k += 1
    for q in range(3):
        out[k] = sl[q]
        k += 1
    return out


def _build_program():
    import concourse.bacc as bacc
    import concourse.mybir as mybir
    import concourse.tile as tile
    from contextlib import ExitStack

    dt = mybir.dt
    Alu = mybir.AluOpType
    Act = mybir.ActivationFunctionType

    nc = bacc.Bacc("TRN2", target_bir_lowering=False, debug=False)

    d_lhsT = nc.declare_dram_parameter("lhsT", [B, K, ROWS], dt.bfloat16, isOutput=False)
    d_rhs_t = nc.declare_dram_parameter("rhs_t", [B, K, N], dt.bfloat16, isOutput=False)
    d_rhs_p = nc.declare_dram_parameter("rhs_p", [B, K, PPW], dt.bfloat16, isOutput=False)
    d_dql = nc.declare_dram_parameter("dql", [128, 128], dt.bfloat16, isOutput=False)
    d_mu = nc.declare_dram_parameter("mu_sl", [1, 128], dt.float32, isOutput=False)
    d_lv = nc.declare_dram_parameter("lv_sl", [1, 128], dt.float32, isOutput=False)

    # o_min: NEGATED row maxes; col 4b+r = pt, col 8+4b+r = pp band
    o_min = nc.declare_dram_parameter("o_min", [128, 16], dt.float32, isOutput=True)
    # o_ct: NEGATED col maxes: pt b: cols 32b..32b+32 (block t);
    #       pp b: cols 64+20b..64+20b+20 (rotated block p)
    o_ct = nc.declare_dram_parameter("o_ct", [128, 104], dt.float32, isOutput=True)
    o_kl = nc.declare_dram_parameter("o_kl", [1, 3], dt.float32, isOutput=True)

    with tile.TileContext(nc) as tc, ExitStack() as ctx:
        consts = ctx.enter_context(tc.tile_pool(name="consts", bufs=1))
        psum = ctx.enter_context(tc.tile_pool(name="psum", bufs=4, space="PSUM"))
        stpool = ctx.enter_context(tc.tile_pool(name="st", bufs=4))
        utpool = ctx.enter_context(tc.tile_pool(name="ut", bufs=2))
        fpool = ctx.enter_context(tc.tile_pool(name="f", bufs=2))

        # ---- resident inputs --------------------------------------------
        lhsT_sb = {}
        rhs_sb = {}
        for b in range(B):
            t1 = consts.tile([K, ROWS], dt.bfloat16, tag=f"l{b}")
            rt = consts.tile([K, N], dt.bfloat16, tag=f"rt{b}")
            rp = consts.tile([K, PPW], dt.bfloat16, tag=f"rp{b}")
            lhsT_sb[b] = t1
            rhs_sb["pt", b] = rt
            rhs_sb["pp", b] = rp
        dql_sb = consts.tile([128, 128], dt.bfloat16, tag="dql")
        mu_sb = consts.tile([1, 128], dt.float32, tag="mu")
        lv_sb = consts.tile([1, 128], dt.float32, tag="lv")

        # input DMAs: the first matmul needs lhsT[0] + rhs_t[0][:, :512] —
        # issue those in PARALLEL on different queues; bulk follows.
        nc.sync.dma_start(out=lhsT_sb[0][:], in_=d_lhsT[0])
        nc.gpsimd.dma_start(out=rhs_sb["pt", 0][:, :512], in_=d_rhs_t[0, :, :512])
        nc.gpsimd.dma_start(out=dql_sb[:], in_=d_dql[:])
        nc.gpsimd.dma_start(out=rhs_sb["pt", 0][:, 512:2048], in_=d_rhs_t[0, :, 512:2048])
        nc.sync.dma_start(out=rhs_sb["pt", 0][:, 2048:], in_=d_rhs_t[0, :, 2048:])
        nc.gpsimd.dma_start(out=rhs_sb["pp", 0][:, :1280], in_=d_rhs_p[0, :, :1280])
        nc.sync.dma_start(out=rhs_sb["pp", 0][:, 1280:], in_=d_rhs_p[0, :, 1280:])
        nc.sync.dma_start(out=lhsT_sb[1][:], in_=d_lhsT[1])
        nc.gpsimd.dma_start(out=rhs_sb["pt", 1][:, :2048], in_=d_rhs_t[1, :, :2048])
        nc.sync.dma_start(out=rhs_sb["pt", 1][:, 2048:], in_=d_rhs_t[1, :, 2048:])
        nc.gpsimd.dma_start(out=rhs_sb["pp", 1][:], in_=d_rhs_p[1])
        nc.gpsimd.dma_start(out=mu_sb[:], in_=d_mu[:])
        nc.gpsimd.dma_start(out=lv_sb[:], in_=d_lv[:])

        omin_sb = consts.tile([128, 16], dt.float32, tag="omin")
        ctall = consts.tile([128, 104], dt.float32, tag="ctall")

        # persistent slabs (negated, max-merged)
        slab = {}
        for b in range(B):
            spt = consts.tile([128, N], dt.bfloat16, tag=f"spt{b}")
            spp = consts.tile([128, PPW], dt.bfloat16, tag=f"spp{b}")
            slab["pt", b] = spt
            slab["pp", b] = spp

        def start_finish(kind, b):
            """Issue the slab transpose DMA; DVE folds come later (deferred
            past the next job so DVE isn't head-of-line blocked on the DMA)."""
            nb = NBLK if kind == "pt" else PPB
            ut = utpool.tile([128, nb, 128], dt.bfloat16, tag=f"ut{kind}")
            nc.sync.dma_start_transpose(ut[:], slab[kind, b][:])
            return ut

        def end_finish(kind, b, ut):
            if kind == "pt":
                nb, off = NBLK, 32 * b
            else:
                nb, off = PPB, 64 + PPB * b
            g1 = fpool.tile([128, nb, 64], dt.bfloat16, tag=f"g1{kind}")
            nc.vector.tensor_tensor(g1[:], ut[:, :, 0:64], ut[:, :, 64:128], Alu.max)
            g2 = fpool.tile([128, nb, 32], dt.bfloat16, tag=f"g2{kind}")
            nc.vector.tensor_tensor(g2[:], g1[:, :, 0:32], g1[:, :, 32:64], Alu.max)
            g3 = fpool.tile([128, nb, 16], dt.bfloat16, tag=f"g3{kind}")
            nc.vector.tensor_tensor(g3[:], g2[:, :, 0:16], g2[:, :, 16:32], Alu.max)
            nc.vector.tensor_reduce(
                ctall[:, off:off + nb], g3[:], axis=mybir.AxisListType.X, op=Alu.max)

        pending = []  # deferred finish chains: (kind, b, ut)
        for b in range(B):
            for kind, r in JOB_ORDER:
                while pending:
                    end_finish(*pending.pop(0))
                lhsT = lhsT_sb[b][:, 128 * r:128 * (r + 1)]
                rhs = rhs_sb[kind, b]
                if kind == "pt":
                    base, w1 = 0, 2048  # second chunk width
                else:
                    base, w1 = 128 * r, 128
                if r == 0:
                    st = slab[kind, b]
                else:
                    st = stpool.tile([128, N if kind == "pt" else BANDW],
                                     dt.bfloat16, tag=f"st{kind}")
                # ---- PSUM chunks of [128,1024] (4-deep pipeline) ---------
                # pt: 4 chunks; pp: 2 chunks + one 128-wide tail chunk.
                widths = [1024, 1024, 1024, 1024] if kind == "pt" else [1024, 1024, 128]
                pos = 0
                for ci, cw_total in enumerate(widths):
                    ch = psum.tile([128, 1024], dt.float32, tag="ps")
                    for t in range(0, cw_total, 512):
                        cw = min(512, cw_total - t)
                        c0 = base + pos + t
                        if kind == "pp" and ci == 0 and t == 0:
                            # diag block [0:128] is its own accumulation group
                            nc.tensor.matmul(
                                ch[:, 0:128], lhsT, rhs[:, c0:c0 + 128],
                                start=True, stop=False)
                            nc.tensor.matmul(
                                ch[:, 0:128], dql_sb[:], dql_sb[:],
                                start=False, stop=True)
                            nc.tensor.matmul(
                                ch[:, 128:512], lhsT, rhs[:, c0 + 128:c0 + 512],
                                start=True, stop=True)
                        else:
                            nc.tensor.matmul(
                                ch[:, t:t + cw], lhsT, rhs[:, c0:c0 + cw],
                                start=True, stop=True)
                    nc.scalar.activation(
                        st[:, pos:pos + cw_total], ch[:, 0:cw_total],
                        Act.Copy, scale=-1.0)
                    pos += cw_total
                # ---- row-max fold chain (TT bf16 2x) + final reduce ------
                col = 4 * b + r if kind == "pt" else 8 + 4 * b + r
                if kind == "pt":
                    f1 = fpool.tile([128, 2048], dt.bfloat16, tag="f1")
                    nc.vector.tensor_tensor(
                        f1[:], st[:, 0:2048], st[:, 2048:4096], Alu.max)
                    f2 = fpool.tile([128, 1024], dt.bfloat16, tag="f2")
                    nc.vector.tensor_tensor(
                        f2[:], f1[:, 0:1024], f1[:, 1024:2048], Alu.max)
                    f3 = fpool.tile([128, 512], dt.bfloat16, tag="f3")
                    nc.vector.tensor_tensor(
                        f3[:], f2[:, 0:512], f2[:, 512:1024], Alu.max)
                    nc.vector.tensor_reduce(
                        omin_sb[:, col:col + 1], f3[:],
                        axis=mybir.AxisListType.X, op=Alu.max)
                else:
                    q1 = fpool.tile([128, 1088], dt.bfloat16, tag="q1")
                    nc.vector.tensor_tensor(
                        q1[:], st[:, 0:1088], st[:, 1088:2176], Alu.max)
                    q2 = fpool.tile([128, 544], dt.bfloat16, tag="q2")
                    nc.vector.tensor_tensor(
                        q2[:], q1[:, 0:544], q1[:, 544:1088], Alu.max)
                    nc.vector.tensor_reduce(
                        omin_sb[:, col:col + 1], q2[:],
                        axis=mybir.AxisListType.X, op=Alu.max)
                # ---- slab merge ------------------------------------------
                if kind == "pp" and r == 0:
                    # init the slab tail the r=0 band doesn't reach
                    nc.vector.memset(slab["pp", b][:, BANDW:PPW], NEG_BIG)
                if r > 0:
                    if kind == "pt":
                        nc.vector.tensor_tensor(
                            slab["pt", b][:], slab["pt", b][:], st[:], Alu.max)
                    else:
                        win = slab["pp", b][:, base:base + BANDW]
                        nc.vector.tensor_tensor(win, win, st[:], Alu.max)
                if (kind, r) in (("pt", 3), ("pp", 3)):
                    pending.append((kind, b, start_finish(kind, b)))
        while pending:
            end_finish(*pending.pop(0))
        # ---- KL partials (at the end: avoids ACT table switches mid-
        # stream; Exp/Square live in a different table set than Copy) ----
        klt = consts.tile([1, 3], dt.float32, tag="klt")
        cp_t = consts.tile([1, 128], dt.float32, tag="klcp")
        nc.scalar.activation(cp_t[:], lv_sb[:], Act.Copy, accum_out=klt[:, 0:1])
        e_t = consts.tile([1, 128], dt.float32, tag="klexp")
        nc.scalar.activation(e_t[:], lv_sb[:], Act.Exp, accum_out=klt[:, 2:3])
        sq_t = consts.tile([1, 128], dt.float32, tag="klsq")
        nc.scalar.activation(sq_t[:], mu_sb[:], Act.Square, accum_out=klt[:, 1:2])

        # ---- outputs (parallel queues to shorten the tail) --------------
        nc.gpsimd.dma_start(out=o_min[:], in_=omin_sb[:])
        nc.sync.dma_start(out=o_ct[:], in_=ctall[:])
        nc.gpsimd.dma_start(out=o_kl[:], in_=klt[:])

    nc.compile()
    return nc


def _make_in_maps(pred, target, mu, logvar):
    pred = np.asarray(pred, dtype=np.float32)
    target = np.asarray(target, dtype=np.float32)
    mu = np.asarray(mu, dtype=np.float32)
    logvar = np.asarray(logvar, dtype=np.float32)

    pred64 = pred.astype(np.float64)
    target64 = target.astype(np.float64)

    rhs_t = np.stack([_build_rhs(target64[b]) for b in range(B)])  # [B,K,N]
    rhs_p_full = np.stack([_build_rhs(pred64[b]) for b in range(B)])
    dql = (np.eye(128) * 1000.0).astype(BF16)
    mu_flat = mu.reshape(-1)
    lv_flat = logvar.reshape(-1)

    in_maps = []
    for c in range(CORES):
        rows = slice(ROWS * c, ROWS * (c + 1))
        lhsT = np.stack([_build_lhsT(pred64[b, rows]) for b in range(B)])
        rot = np.roll(rhs_p_full, -ROWS * c, axis=2)[:, :, :PPW]
        in_maps.append({
            "lhsT": lhsT,
            "rhs_t": rhs_t,
            "rhs_p": np.ascontiguousarray(rot),
            "dql": dql,
            "mu_sl": mu_flat[128 * c:128 * (c + 1)].reshape(1, 128),
            "lv_sl": lv_flat[128 * c:128 * (c + 1)].reshape(1, 128),
        })
    return in_maps


def _host_post(results):
    # row mins (negated maxes): pt cols 0..7, pp cols 8..15
    nn_pt = np.empty((B, N), dtype=np.float64)
    pp_row = np.empty((B, N), dtype=np.float64)
    for c in range(CORES):
        om = -results[c]["o_min"].astype(np.float64)  # [128, 16]
        for b in range(B):
            for r in range(RB):
                rows = slice(ROWS * c + 128 * r, ROWS * c + 128 * r + 128)
                nn_pt[b, rows] = om[:, 4 * b + r]
                pp_row[b, rows] = om[:, 8 + 4 * b + r]

    # col mins: o_ct [128, 104] negated
    cts = -np.stack([r["o_ct"] for r in results]).astype(np.float64)
    nn_tp = np.full((B, N), np.inf)
    pp_col = np.full((B, N), np.inf)
    for c in range(CORES):
        for b in range(B):
            pt_v = cts[c][:, 32 * b:32 * b + 32]  # [jrel, t]
            nn_tp[b] = np.minimum(nn_tp[b], pt_v.T.reshape(N))
            pp_v = cts[c][:, 64 + PPB * b:64 + PPB * b + PPB]  # [jrel, p]
            for p in range(PPB):
                gblk = (4 * c + p) % NBLK
                cols = slice(128 * gblk, 128 * gblk + 128)
                pp_col[b, cols] = np.minimum(pp_col[b, cols], pp_v[:, p])
    nn_pp = np.minimum(pp_row, pp_col)

    kl_parts = np.stack([r["o_kl"].reshape(3) for r in results])

    cd = (nn_pt.mean(axis=1) + nn_tp.mean(axis=1)).mean()

    s1 = kl_parts[:, 0].astype(np.float64).sum()
    s2 = kl_parts[:, 1].astype(np.float64).sum()
    s3 = kl_parts[:, 2].astype(np.float64).sum()
    n_kl = B * L
    kl = -0.5 * (n_kl + s1 - s2 - s3) / n_kl

    density = np.std(nn_pp, axis=1, ddof=1).mean()

    total = cd + 0.001 * kl + 0.1 * density

    return (
        np.float32(total),
        np.float32(cd),
        np.float32(kl),
        np.float32(density),
    )


def kernel(pred, target, mu, logvar):
    from concourse.bass_utils import run_bass_kernel_spmd

    in_maps = _make_in_maps(pred, target, mu, logvar)
    nc = _build_program()
    res = run_bass_kernel_spmd(nc, in_maps, list(range(CORES)))
    return _host_post(res.results)
